# revision 2
# baseline (speedup 1.0000x reference)
"""Dense dot-product attention with key-length masking on 8 Trainium2 cores.

Problem: q,k,v [16, 2048, 128] fp32, valid_lens [16,1] int32.
  out = softmax(mask(q@k.T/sqrt(d))) @ v   (masked keys -> -1e6 before softmax)

v1 design (from trace analysis of the 58.5us baseline):
- The baseline was co-bottlenecked by the ACT engine (exp: 47us) and the
  PE (46us), with 37 key tiles/core scheduled vs the 31.75 ideal.
- Work unit = (batch, 1024-query slice, key-tile range).  Since softmax
  denominators are finished on the host, a unit can be SPLIT along keys:
  partial O sums and partial E sums combine on the host.  This removes
  the group-of-8 quantization: slots of sizes SLOT_T (waste 2/256).
- exp is split across TWO engines: ACT does most tiles exactly;
  DVE computes exp via the Schraudolph bit trick in one tensor_scalar:
  int16(round(S*A_k + B_k)) bitcast to fp16 ~= exp(scale*S+bias), with
  per-partition A,B folding in the key mask (masked -> bits 0 -> E=0).
- Denominator tree adds are split DVE/Pool (Pool is idle otherwise).
- HAM warm-up retained: dummy bf16 matmuls lift the PE clock-gate while
  input DMAs stream.
"""

import math
import sys
import types

import numpy as np

import concourse.bass as bass
import concourse.mybir as mybir
import concourse.tile as tile
from concourse.tile import add_dep_helper
from concourse import bacc
from concourse.bass_utils import run_bass_kernel_spmd

B, Q, K, D = 16, 2048, 2048, 128
NCORES = 8
QCH = 1024         # queries per work unit
MM_N = 512         # moving-operand free dim per matmul
KT = K // 128      # max key tiles per batch
SCALE = 1.0 / math.sqrt(D)
NEG_BIAS = -30.0   # ACT path: exp(-30) ~ 1e-13
WARMUP_MMS = 7

# Schraudolph fp16 exp: bits = round(x * 1024/ln2 + B); B centered so the
# log-linear interpolation error is ~symmetric (+-3%).
SCHR_A = 1024.0 / math.log(2.0)
SCHR_B = 15360.0 - 44.5

F32 = mybir.dt.float32
F16 = mybir.dt.float16
I16 = mybir.dt.int16
BF16 = mybir.dt.bfloat16

DVE_FRAC = 0.35    # fraction of each slot's exp tiles on DVE
POOL_ADD_FRAC = 0.20  # fraction of tree level-1 adds on Pool


def _install_hook_stub():
    if "antenv.axon_hooks" in sys.modules:
        return
    mod = types.ModuleType("antenv.axon_hooks")
    _hook = [None]
    mod.set_axon_ntff_profile_hook = lambda h: _hook.__setitem__(0, h)
    mod.get_axon_ntff_profile_hook = lambda: _hook[0]
    sys.modules["antenv.axon_hooks"] = mod


_install_hook_stub()

_build_cache = {}
last_result = None


def _dve_set(t):
    """Which tile indices of a t-tile slot run exp on DVE."""
    nd = int(round(t * DVE_FRAC))
    if nd == 0:
        return frozenset()
    # spread through the slot, avoid tile 0 (pipeline ignition on ACT)
    step = t / nd
    return frozenset(min(t - 1, 1 + int(j * step)) for j in range(nd))


def _pool_adds(t):
    """Which level-1 tree add indices run on Pool (early pairs finish first)."""
    n1 = t // 2
    np_ = int(round(n1 * POOL_ADD_FRAC * 2.0))
    return frozenset(range(min(np_, n1)))


def _build(slot_ts):
    """One SPMD program: slot j processes slot_ts[j] key tiles of one
    (batch, q-slice, key-range) sub-unit."""
    nslot = len(slot_ts)
    tmax = max(slot_ts)
    nc = bacc.Bacc(num_devices=NCORES)

    qT = nc.declare_dram_parameter("qT", [nslot, D, QCH], F16, isOutput=False)
    kT = nc.declare_dram_parameter("kT", [nslot, D, tmax * 128], F16, isOutput=False)
    v = nc.declare_dram_parameter("v", [nslot, tmax * 128, D], F16, isOutput=False)
    sc = nc.declare_dram_parameter("sc", [nslot, 128, KT], F32, isOutput=False)
    bi = nc.declare_dram_parameter("bi", [nslot, 128, KT], F32, isOutput=False)
    oT = nc.declare_dram_parameter("oT", [nslot, D, QCH], F32, isOutput=True)
    esum = nc.declare_dram_parameter("esum", [nslot, 128, QCH], F16, isOutput=True)

    with tile.TileContext(nc) as tc:
        with (
            tc.tile_pool(name="consts", bufs=1) as consts,
            tc.tile_pool(name="inputs", bufs=2) as inpool,
            tc.tile_pool(name="epool", bufs=tmax + 8) as epool,
            tc.tile_pool(name="treep", bufs=3) as treepool,
            tc.tile_pool(name="osb", bufs=2) as opool,
            tc.tile_pool(name="sps", bufs=3, space="PSUM") as pspool,
            tc.tile_pool(name="oacc", bufs=1, space="PSUM") as psacc,
        ):
            # --- HAM warm-up ---
            wsrc = consts.tile([128, MM_N], BF16)
            nc.vector.memset(wsrc[:], 1.0)
            for w in range(WARMUP_MMS):
                if w % 2 == 0:
                    wps = pspool.tile([128, QCH], F32, tag="s")
                nc.tensor.matmul(
                    wps[:, (w % 2) * MM_N : (w % 2) * MM_N + MM_N],
                    wsrc[:, :128],
                    wsrc[:],
                    start=True,
                    stop=True,
                    skip_group_check=True,
                )

            prev_in_dmas = []
            for s in range(nslot):
                t = slot_ts[s]
                dset = _dve_set(t)
                padds = _pool_adds(t)
                qT_sb = inpool.tile([128, QCH], F16, tag="qT")
                kT_sb = inpool.tile([128, t * 128], F16, tag="kT")
                v_sb = inpool.tile([128, t, D], F16, tag="v")
                sc_sb = inpool.tile([128, KT], F32, tag="sc")
                bi_sb = inpool.tile([128, KT], F32, tag="bi")
                pk = 4
                in_dmas = []
                nq = 2 if s == 0 else 1
                for j in range(nq):
                    eng = nc.sync if not (s == 0 and j == 1) else nc.scalar
                    d = eng.dma_start(
                        out=qT_sb[:, bass.ts(j, QCH // nq)],
                        in_=qT[s][:, bass.ts(j, QCH // nq)],
                    )
                    in_dmas.append(d)
                v_dram = v[s].rearrange("(i p) d -> p i d", p=128)
                ntp = (t + pk - 1) // pk
                for j in range(ntp):
                    klo, khi = j * pk * 128, min(t * 128, (j + 1) * pk * 128)
                    keng = nc.gpsimd if (s == 0 and j == 0) else nc.sync
                    in_dmas.append(
                        keng.dma_start(out=kT_sb[:, klo:khi], in_=kT[s][:, klo:khi])
                    )
                    tlo, thi = j * pk, min(t, (j + 1) * pk)
                    in_dmas.append(
                        nc.sync.dma_start(out=v_sb[:, tlo:thi, :], in_=v_dram[:, tlo:thi, :])
                    )
                nc.sync.dma_start(out=sc_sb[:], in_=sc[s])
                nc.sync.dma_start(out=bi_sb[:], in_=bi[s])
                if s == 1:
                    for p in prev_in_dmas:
                        add_dep_helper(
                            in_dmas[0].ins, p.ins,
                            reason="slot0 input DMA priority",
                        )
                prev_in_dmas = in_dmas

                etiles = []
                o_ps = psacc.tile([128, QCH], F32, tag="o")
                for i in range(t):
                    s_ps = pspool.tile([128, QCH], F32, tag="s")
                    for h in range(QCH // MM_N):
                        nc.tensor.matmul(
                            s_ps[:, bass.ts(h, MM_N)],
                            kT_sb[:, bass.ts(i, 128)],
                            qT_sb[:, bass.ts(h, MM_N)],
                            start=True,
                            stop=True,
                        )
                    e_sb = epool.tile([128, QCH], F16, tag="e")
                    etiles.append(e_sb)
                    if i in dset:
                        # Schraudolph exp on DVE: int16 bits of fp16
                        nc.vector.tensor_scalar(
                            e_sb[:].bitcast(I16),
                            s_ps[:],
                            sc_sb[:, i : i + 1],
                            bi_sb[:, i : i + 1],
                            mybir.AluOpType.mult,
                            mybir.AluOpType.add,
                        )
                    else:
                        parts = (
                            [bass.ts(p, MM_N) for p in range(2)]
                            if (s == 0 and i == 0)
                            else [slice(None)]
                        )
                        for pr in parts:
                            nc.scalar.activation(
                                e_sb[:, pr],
                                s_ps[:, pr],
                                mybir.ActivationFunctionType.Exp,
                                bias=bi_sb[:, i : i + 1],
                                scale=sc_sb[:, i : i + 1],
                            )
                    for h in range(QCH // MM_N):
                        nc.tensor.matmul(
                            o_ps[:, bass.ts(h, MM_N)],
                            v_sb[:, i, :],
                            e_sb[:, bass.ts(h, MM_N)],
                            start=(i == 0),
                            stop=(i == t - 1),
                        )

                # denominator: pairwise fp16 tree; level-1 split DVE/Pool
                cur = [e[:] for e in etiles]
                if len(cur) > 1:
                    tr = treepool.tile([128, (t + 1) // 2, QCH], F16, tag="tr")
                    nxt = []
                    for j in range(len(cur) // 2):
                        eng = nc.gpsimd if j in padds else nc.vector
                        eng.tensor_add(tr[:, j, :], cur[2 * j], cur[2 * j + 1])
                        nxt.append(tr[:, j, :])
                    if len(cur) % 2:
                        nxt.append(cur[-1])
                    cur = nxt
                    while len(cur) > 1:
                        nxt = []
                        for j in range(len(cur) // 2):
                            nc.vector.tensor_add(cur[2 * j], cur[2 * j], cur[2 * j + 1])
                            nxt.append(cur[2 * j])
                        if len(cur) % 2:
                            nxt.append(cur[-1])
                        cur = nxt
                nc.sync.dma_start(out=esum[s], in_=cur[0])

                o_sb = opool.tile([128, QCH], F32, tag="osb")
                for h in range(2):
                    hs = bass.ts(h, QCH // 2)
                    nc.vector.tensor_copy(o_sb[:, hs], o_ps[:, hs])
                    nc.sync.dma_start(out=oT[s][:, hs], in_=o_sb[:, hs])

    nc.compile()
    return nc


def _schedule(need):
    """Pack 32 (batch, q-half) units of sizes need[] into 8 cores x slots,
    each bin = one key-range of one unit.  Returns (slot_ts, assign) where
    assign[core][slot] = (b, h, t0, nt)."""
    units = [[int(need[b]), b, h] for b in range(B) for h in range(Q // QCH)]
    total = sum(u[0] for u in units)

    def pack(T):
        bins = []
        for t in sorted(T, reverse=True):
            bins += [t] * NCORES
        order = sorted(range(len(bins)), key=lambda i: -bins[i])
        avail = list(order)
        placed = {}  # bin idx -> (b, h, t0, nt)
        pool = sorted(([n, b, h, 0] for n, b, h in units), reverse=True)
        while pool:
            n, b, h, t0 = pool.pop(0)
            if n == 0:
                continue
            exact = [i for i in avail if bins[i] == n]
            if exact:
                i = exact[0]
            else:
                geq = sorted((i for i in avail if bins[i] > n), key=lambda i: bins[i])
                if geq:
                    i = geq[0]
                else:
                    if not avail:
                        return None, None
                    i = max(avail, key=lambda j: bins[j])
            avail.remove(i)
            nt = min(n, bins[i])
            placed[i] = (b, h, t0, nt)
            if nt < n:
                pool.append([n - nt, b, h, t0 + nt])
                pool.sort(reverse=True)
        waste = sum(bins[i] for i in avail) + sum(
            bins[i] - placed[i][3] for i in placed
        )
        return waste, (bins, placed)

    # candidate slot-size vectors
    import itertools

    best = None
    lo = (total + NCORES - 1) // NCORES
    for ns in (5, 6):
        for T in itertools.combinations_with_replacement(range(1, KT + 1), ns):
            st = sum(T)
            if not (lo <= st <= lo + 3):
                continue
            w, res = pack(tuple(sorted(T, reverse=True)))
            if w is None:
                continue
            key = (st, w)
            if best is None or key < best[0]:
                best = (key, tuple(sorted(T, reverse=True)), res)
    (st, w), T, (bins, placed) = best

    # order slots: second-smallest first, smallest last, rest descending
    sizes = sorted(set(T))
    Tsorted = sorted(T, reverse=True)
    if len(Tsorted) >= 2:
        slot_order_sizes = [Tsorted[-2]] + [
            x for i, x in enumerate(Tsorted) if i not in (len(Tsorted) - 1, len(Tsorted) - 2)
        ] + [Tsorted[-1]]
    else:
        slot_order_sizes = Tsorted
    # bins are grouped: first 8 bins = largest size, etc. (sorted desc)
    size_to_binidxs = {}
    idx = 0
    for t in Tsorted:
        size_to_binidxs.setdefault(t, []).append(list(range(idx, idx + NCORES)))
        idx += NCORES
    assign = [[None] * len(Tsorted) for _ in range(NCORES)]
    slot_ts = []
    used_groups = {t: 0 for t in size_to_binidxs}
    for sidx, t in enumerate(slot_order_sizes):
        grp = size_to_binidxs[t][used_groups[t]]
        used_groups[t] += 1
        slot_ts.append(t)
        for c in range(NCORES):
            i = grp[c]
            assign[c][sidx] = placed.get(i)  # None for padded-empty bin
    return tuple(slot_ts), assign


def kernel(q, k, v, valid_lens):
    q = np.ascontiguousarray(q, dtype=np.float32)
    k = np.ascontiguousarray(k, dtype=np.float32)
    v = np.ascontiguousarray(v, dtype=np.float32)
    L = np.asarray(valid_lens).reshape(-1).astype(np.int64)

    need = np.where(L == 0, KT, np.minimum(KT, (L + 127) // 128)).astype(np.int64)
    slot_ts, assign = _schedule(need)

    if slot_ts not in _build_cache:
        _build_cache[slot_ts] = _build(slot_ts)
    nc = _build_cache[slot_ts]

    qh = q.astype(np.float16)
    kh = k.astype(np.float16)
    vh = v.astype(np.float16)
    kidx = np.arange(128)
    tmax = max(slot_ts)

    in_maps = []
    for c in range(NCORES):
        qT_a = np.zeros((len(slot_ts), D, QCH), np.float16)
        kT_a = np.zeros((len(slot_ts), D, tmax * 128), np.float16)
        v_a = np.zeros((len(slot_ts), tmax * 128, D), np.float16)
        sc_a = np.zeros((len(slot_ts), 128, KT), np.float32)
        bi_a = np.zeros((len(slot_ts), 128, KT), np.float32)
        for s, t in enumerate(slot_ts):
            dset = _dve_set(t)
            part = assign[c][s]
            # default: all tiles padded-off (E=0)
            for i in range(t):
                if i in dset:
                    sc_a[s, :, i] = 0.0
                    bi_a[s, :, i] = 0.0
                else:
                    sc_a[s, :, i] = 0.0
                    bi_a[s, :, i] = NEG_BIAS
            if part is None:
                continue
            b, h, t0, nt = part
            lb = int(L[b])
            qT_a[s] = qh[b, h * QCH : (h + 1) * QCH].T
            kT_a[s, :, : nt * 128] = kh[b].T[:, t0 * 128 : (t0 + nt) * 128]
            v_a[s, : nt * 128] = vh[b, t0 * 128 : (t0 + nt) * 128]
            for i in range(nt):
                g = t0 + i  # global key tile
                if lb == 0:
                    m = None  # uniform: E=1 on every key
                else:
                    m = (g * 128 + kidx < lb).astype(np.float32)
                if i in dset:
                    if m is None:
                        sc_a[s, :, i] = 0.0
                        bi_a[s, :, i] = SCHR_B + 44.5  # exactly 15360 -> E=1.0
                    else:
                        sc_a[s, :, i] = m * np.float32(SCALE * SCHR_A)
                        bi_a[s, :, i] = m * np.float32(SCHR_B)
                else:
                    if m is None:
                        sc_a[s, :, i] = 0.0
                        bi_a[s, :, i] = 0.0
                    else:
                        sc_a[s, :, i] = m * np.float32(SCALE)
                        bi_a[s, :, i] = (1.0 - m) * np.float32(NEG_BIAS)
        in_maps.append({"qT": qT_a, "kT": kT_a, "v": v_a, "sc": sc_a, "bi": bi_a})

    res = run_bass_kernel_spmd(nc, in_maps, list(range(NCORES)))
    global last_result
    last_result = res

    acc = np.zeros((B, Q // QCH, D, QCH), np.float64)
    den = np.zeros((B, Q // QCH, QCH), np.float64)
    for c in range(NCORES):
        r = res.results[c]
        for s in range(len(slot_ts)):
            part = assign[c][s]
            if part is None:
                continue
            b, h, _, _ = part
            acc[b, h] += r["oT"][s].astype(np.float64)
            den[b, h] += r["esum"][s].astype(np.float64).sum(axis=0)
    out = np.empty((B, Q, D), np.float32)
    for b in range(B):
        for h in range(Q // QCH):
            out[b, h * QCH : (h + 1) * QCH] = (acc[b, h] / den[b, h][None, :]).T
    return out


# revision 26
# speedup vs baseline: 1.1751x; 1.1751x over previous
"""Dense dot-product attention with key-length masking on 8 Trainium2 cores.

Problem: q,k,v [16, 2048, 128] fp32, valid_lens [16,1] int32.
  out = softmax(mask(q@k.T/sqrt(d))) @ v   (masked keys -> -1e6 before softmax)

v1 design (from trace analysis of the 58.5us baseline):
- The baseline was co-bottlenecked by the ACT engine (exp: 47us) and the
  PE (46us), with 37 key tiles/core scheduled vs the 31.75 ideal.
- Work unit = (batch, 1024-query slice, key-tile range).  Since softmax
  denominators are finished on the host, a unit can be SPLIT along keys:
  partial O sums and partial E sums combine on the host.  This removes
  the group-of-8 quantization: slots of sizes SLOT_T (waste 2/256).
- exp is split across TWO engines: ACT does most tiles exactly;
  DVE computes exp via the Schraudolph bit trick in one tensor_scalar:
  int16(round(S*A_k + B_k)) bitcast to fp16 ~= exp(scale*S+bias), with
  per-partition A,B folding in the key mask (masked -> bits 0 -> E=0).
- Denominator tree adds are split DVE/Pool (Pool is idle otherwise).
- HAM warm-up retained: dummy bf16 matmuls lift the PE clock-gate while
  input DMAs stream.
"""

import math
import sys
import types

import numpy as np

import concourse.bass as bass
import concourse.mybir as mybir
import concourse.tile as tile
from concourse.tile import add_dep_helper
from concourse import bacc
from concourse.bass_utils import run_bass_kernel_spmd

B, Q, K, D = 16, 2048, 2048, 128
NCORES = 8
QCH = 1024         # queries per work unit
MM_N = 512         # moving-operand free dim per matmul
KT = K // 128      # max key tiles per batch
SCALE = 1.0 / math.sqrt(D)
NEG_BIAS = -30.0   # ACT path: exp(-30) ~ 1e-13
WARMUP_MMS = 7

# Schraudolph fp16 exp: bits = round(x * 1024/ln2 + B); B centered so the
# log-linear interpolation error is ~symmetric (+-3%).
SCHR_A = 1024.0 / math.log(2.0)
SCHR_B = 15360.0 - 44.5

F32 = mybir.dt.float32
F16 = mybir.dt.float16
I16 = mybir.dt.int16
BF16 = mybir.dt.bfloat16

FP8 = mybir.dt.float8e4
DR = mybir.MatmulPerfMode.DoubleRow
U16 = mybir.dt.uint16
DVE_FRAC = 0.15    # fraction of each slot's exp tiles on DVE (tail block)
# Global logit shift: keeps exp() under the fp8e4 max (240) for the observed
# logit range (max ~7.5), while the Schraudolph uint16 floor (saturate-to-0)
# covers logits below -10.4+2.5.
SHIFT = -2.5


def _install_hook_stub():
    if "antenv.axon_hooks" in sys.modules:
        return
    mod = types.ModuleType("antenv.axon_hooks")
    _hook = [None]
    mod.set_axon_ntff_profile_hook = lambda h: _hook.__setitem__(0, h)
    mod.get_axon_ntff_profile_hook = lambda: _hook[0]
    sys.modules["antenv.axon_hooks"] = mod


_install_hook_stub()

_build_cache = {}
last_result = None


def _dve_set(t):
    """Which tile indices of a t-tile slot run exp on DVE: a tail block, so
    the ACT prefix can pair into fp8 DoubleRow O-matmuls and padding tiles
    (at the end of underfilled bins) land on DVE's exact E=0 path."""
    nd = int(round(t * DVE_FRAC))
    return frozenset(range(t - nd, t))


def _slot_plan(t):
    """Per-slot structure: ACT prefix tiles pair up for fp8 DR; returns
    (nd, pairs, singles) with pairs = [(2j, 2j+1), ...] fp8 tiles and
    singles = leftover-ACT + DVE tiles kept fp16."""
    nd = int(round(t * DVE_FRAC))
    na = t - nd
    pairs = []  # fp8 DR disabled: e4m3 weight steps cost too much accuracy
    singles = list(range(t))
    return nd, pairs, singles


def _build(slot_ts):
    """One SPMD program: slot j processes slot_ts[j] key tiles of one
    (batch, q-slice, key-range) sub-unit."""
    nslot = len(slot_ts)
    tmax = max(slot_ts)
    nc = bacc.Bacc(num_devices=NCORES)

    qT = nc.declare_dram_parameter("qT", [nslot, D, QCH], F16, isOutput=False)
    kT = nc.declare_dram_parameter("kT", [nslot, D, tmax * 128], F16, isOutput=False)
    v = nc.declare_dram_parameter("v", [nslot, tmax * 128, D], F16, isOutput=False)
    sc = nc.declare_dram_parameter("sc", [nslot, 128, KT], F32, isOutput=False)
    bi = nc.declare_dram_parameter("bi", [nslot, 128, KT], F32, isOutput=False)
    oT = nc.declare_dram_parameter("oT", [nslot, D, QCH], F32, isOutput=True)
    esum = nc.declare_dram_parameter("esum", [nslot, 128, QCH], F16, isOutput=True)

    with tile.TileContext(nc) as tc:
        with (
            tc.tile_pool(name="consts", bufs=1) as consts,
            tc.tile_pool(name="inputs", bufs=2) as inpool,
            tc.tile_pool(name="epool", bufs=tmax + 8) as epool,
            tc.tile_pool(name="e8pool", bufs=tmax // 2 + 4) as e8pool,
            tc.tile_pool(name="treep", bufs=3) as treepool,
            tc.tile_pool(name="osb", bufs=2) as opool,
            tc.tile_pool(name="sps", bufs=2, space="PSUM") as pspool,
            tc.tile_pool(name="oacc", bufs=2, space="PSUM") as psacc,
        ):
            # --- HAM warm-up ---
            wsrc = consts.tile([128, MM_N], BF16)
            nc.vector.memset(wsrc[:], 1.0)
            for w in range(WARMUP_MMS):
                if w % 2 == 0:
                    wps = pspool.tile([128, QCH], F32, tag="s")
                nc.tensor.matmul(
                    wps[:, (w % 2) * MM_N : (w % 2) * MM_N + MM_N],
                    wsrc[:, :128],
                    wsrc[:],
                    start=True,
                    stop=True,
                    skip_group_check=True,
                )

            prev_in_dmas = []
            for s in range(nslot):
                t = slot_ts[s]
                nd, prs, singles = _slot_plan(t)
                qT_sb = inpool.tile([128, QCH], F16, tag="qT")
                kT_sb = inpool.tile([128, t * 128], F16, tag="kT")
                v_sb = inpool.tile([128, t, D], F16, tag="v")
                sc_sb = inpool.tile([128, KT], F32, tag="sc")
                bi_sb = inpool.tile([128, KT], F32, tag="bi")
                pk = 4
                in_dmas = []
                nq = 2 if s == 0 else 1
                for j in range(nq):
                    eng = nc.sync if not (s == 0 and j == 1) else nc.scalar
                    d = eng.dma_start(
                        out=qT_sb[:, bass.ts(j, QCH // nq)],
                        in_=qT[s][:, bass.ts(j, QCH // nq)],
                    )
                    in_dmas.append(d)
                v_dram = v[s].rearrange("(i p) d -> p i d", p=128)
                ntp = (t + pk - 1) // pk
                for j in range(ntp):
                    klo, khi = j * pk * 128, min(t * 128, (j + 1) * pk * 128)
                    keng = nc.gpsimd if (s == 0 and j == 0) else nc.sync
                    in_dmas.append(
                        keng.dma_start(out=kT_sb[:, klo:khi], in_=kT[s][:, klo:khi])
                    )
                    tlo, thi = j * pk, min(t, (j + 1) * pk)
                    in_dmas.append(
                        nc.sync.dma_start(out=v_sb[:, tlo:thi, :], in_=v_dram[:, tlo:thi, :])
                    )
                nc.sync.dma_start(out=sc_sb[:], in_=sc[s])
                nc.sync.dma_start(out=bi_sb[:], in_=bi[s])
                if s == 1:
                    for p in prev_in_dmas:
                        add_dep_helper(
                            in_dmas[0].ins, p.ins,
                            reason="slot0 input DMA priority",
                        )
                prev_in_dmas = in_dmas

                # per-tile E storage: fp8 pair tiles for the ACT prefix,
                # fp16 singles for leftover-ACT and DVE tiles
                # pair tile layout [128, h-half, pair-half, 512]: keeps the DR
                # moving operand e8t[:, h] == [128, 2, 512] contiguous
                pair_of = {}
                for pj, (a, b) in enumerate(prs):
                    e8t = e8pool.tile([128, 2, 2, MM_N], FP8, tag="e8")
                    pair_of[a] = (e8t, 0, pj)
                    pair_of[b] = (e8t, 1, pj)
                esingle = {}

                o_ps = psacc.tile([128, QCH], F32, tag="o")
                n_osrc = len(prs) + len(singles)  # O-matmul groups
                oidx = 0
                for i in range(t):
                    s_ps = pspool.tile([128, QCH], F32, tag="s")
                    for h in range(QCH // MM_N):
                        nc.tensor.matmul(
                            s_ps[:, bass.ts(h, MM_N)],
                            kT_sb[:, bass.ts(i, 128)],
                            qT_sb[:, bass.ts(h, MM_N)],
                            start=True,
                            stop=True,
                        )
                    if i >= t - nd:
                        # Schraudolph exp on DVE -> fp16 single
                        e_sb = epool.tile([128, QCH], F16, tag="e")
                        esingle[i] = e_sb
                        nc.vector.tensor_scalar(
                            e_sb[:].bitcast(U16),
                            s_ps[:],
                            sc_sb[:, i : i + 1],
                            bi_sb[:, i : i + 1],
                            mybir.AluOpType.mult,
                            mybir.AluOpType.add,
                        )
                    elif i in pair_of:
                        e8t, half, _ = pair_of[i]
                        # one activation per 512-column half: contiguous out
                        for h in range(QCH // MM_N):
                            nc.scalar.activation(
                                e8t[:, h, half, :],
                                s_ps[:, bass.ts(h, MM_N)],
                                mybir.ActivationFunctionType.Exp,
                                bias=bi_sb[:, i : i + 1],
                                scale=sc_sb[:, i : i + 1],
                            )
                    else:
                        e_sb = epool.tile([128, QCH], F16, tag="e")
                        esingle[i] = e_sb
                        parts = (
                            [bass.ts(p, MM_N) for p in range(2)]
                            if (s == 0 and i == 0)
                            else [slice(None)]
                        )
                        for pr in parts:
                            nc.scalar.activation(
                                e_sb[:, pr],
                                s_ps[:, pr],
                                mybir.ActivationFunctionType.Exp,
                                bias=bi_sb[:, i : i + 1],
                                scale=sc_sb[:, i : i + 1],
                            )
                    # O accumulation: fp8 DR when tile closes a pair; fp16
                    # single otherwise
                    if i in pair_of and pair_of[i][1] == 1:
                        e8t, _, pj = pair_of[i]
                        a = 2 * pj
                        for h in range(QCH // MM_N):
                            nc.tensor.matmul(
                                o_ps[:, bass.ts(h, MM_N)],
                                v8_sb[:, a : a + 2, :],
                                e8t[:, h],
                                start=(oidx == 0),
                                stop=(oidx == n_osrc - 1),
                                perf_mode=DR,
                            )
                        oidx += 1
                    elif i in esingle:
                        e_sb = esingle[i]
                        for h in range(QCH // MM_N):
                            nc.tensor.matmul(
                                o_ps[:, bass.ts(h, MM_N)],
                                v_sb[:, i, :],
                                e_sb[:, bass.ts(h, MM_N)],
                                start=(oidx == 0),
                                stop=(oidx == n_osrc - 1),
                            )
                        oidx += 1

                # denominator tree: fp8 pair adds (Pool/DVE alternating),
                # then fp16 pairwise on DVE
                cur = []
                tr = treepool.tile([128, max(1, len(prs) + (len(singles) + 1) // 2), QCH],
                                   F16, tag="tr")
                trj = 0
                for pj, (a, b) in enumerate(prs):
                    e8t = pair_of[a][0]
                    eng = nc.gpsimd if pj % 2 == 0 else nc.vector
                    for h in range(QCH // MM_N):
                        eng.tensor_add(
                            tr[:, trj, bass.ts(h, MM_N)], e8t[:, h, 0, :], e8t[:, h, 1, :]
                        )
                    cur.append(tr[:, trj, :])
                    trj += 1
                sing = [esingle[i][:] for i in sorted(esingle)]
                for j in range(0, len(sing) - 1, 2):
                    nc.vector.tensor_add(tr[:, trj, :], sing[j], sing[j + 1])
                    cur.append(tr[:, trj, :])
                    trj += 1
                if len(sing) % 2:
                    cur.append(sing[-1])
                while len(cur) > 1:
                    nxt = []
                    for j in range(len(cur) // 2):
                        nc.vector.tensor_add(cur[2 * j], cur[2 * j], cur[2 * j + 1])
                        nxt.append(cur[2 * j])
                    if len(cur) % 2:
                        nxt.append(cur[-1])
                    cur = nxt
                nc.sync.dma_start(out=esum[s], in_=cur[0])

                # PSUM -> SBUF -> DRAM; copies split across ACT and DVE
                o_sb = opool.tile([128, QCH], F32, tag="osb")
                for h in range(2):
                    hs = bass.ts(h, QCH // 2)
                    if h == 0:
                        nc.scalar.copy(o_sb[:, hs], o_ps[:, hs])
                    else:
                        nc.vector.tensor_copy(o_sb[:, hs], o_ps[:, hs])
                    nc.sync.dma_start(out=oT[s][:, hs], in_=o_sb[:, hs])

    nc.compile()
    return nc


def _schedule(need):
    """Pack 32 (batch, q-half) units of sizes need[] into 8 cores x slots,
    each bin = one key-range of one unit.  Returns (slot_ts, assign) where
    assign[core][slot] = (b, h, t0, nt)."""
    units = [[int(need[b]), b, h] for b in range(B) for h in range(Q // QCH)]
    total = sum(u[0] for u in units)

    def pack(T):
        bins = []
        for t in sorted(T, reverse=True):
            bins += [t] * NCORES
        order = sorted(range(len(bins)), key=lambda i: -bins[i])
        avail = list(order)
        placed = {}  # bin idx -> (b, h, t0, nt)
        pool = sorted(([n, b, h, 0] for n, b, h in units), reverse=True)
        while pool:
            n, b, h, t0 = pool.pop(0)
            if n == 0:
                continue
            exact = [i for i in avail if bins[i] == n]
            if exact:
                i = exact[0]
            else:
                geq = sorted((i for i in avail if bins[i] > n), key=lambda i: bins[i])
                if geq:
                    i = geq[0]
                else:
                    if not avail:
                        return None, None
                    i = max(avail, key=lambda j: bins[j])
            avail.remove(i)
            nt = min(n, bins[i])
            placed[i] = (b, h, t0, nt)
            if nt < n:
                pool.append([n - nt, b, h, t0 + nt])
                pool.sort(reverse=True)
        waste = sum(bins[i] for i in avail) + sum(
            bins[i] - placed[i][3] for i in placed
        )
        return waste, (bins, placed)

    # candidate slot-size vectors
    import itertools

    best = None
    lo = (total + NCORES - 1) // NCORES
    for ns in (5, 6):
        for T in itertools.combinations_with_replacement(range(1, KT + 1), ns):
            st = sum(T)
            if not (lo <= st <= lo + 3):
                continue
            w, res = pack(tuple(sorted(T, reverse=True)))
            if w is None:
                continue
            key = (st, w)
            if best is None or key < best[0]:
                best = (key, tuple(sorted(T, reverse=True)), res)
    (st, w), T, (bins, placed) = best

    # order slots: second-smallest first, smallest last, rest descending
    sizes = sorted(set(T))
    Tsorted = sorted(T, reverse=True)
    if len(Tsorted) >= 2:
        slot_order_sizes = [Tsorted[-2]] + [
            x for i, x in enumerate(Tsorted) if i not in (len(Tsorted) - 1, len(Tsorted) - 2)
        ] + [Tsorted[-1]]
    else:
        slot_order_sizes = Tsorted
    # bins are grouped: first 8 bins = largest size, etc. (sorted desc)
    size_to_binidxs = {}
    idx = 0
    for t in Tsorted:
        size_to_binidxs.setdefault(t, []).append(list(range(idx, idx + NCORES)))
        idx += NCORES
    assign = [[None] * len(Tsorted) for _ in range(NCORES)]
    slot_ts = []
    used_groups = {t: 0 for t in size_to_binidxs}
    for sidx, t in enumerate(slot_order_sizes):
        grp = size_to_binidxs[t][used_groups[t]]
        used_groups[t] += 1
        slot_ts.append(t)
        for c in range(NCORES):
            i = grp[c]
            assign[c][sidx] = placed.get(i)  # None for padded-empty bin
    return tuple(slot_ts), assign


def kernel(q, k, v, valid_lens):
    q = np.ascontiguousarray(q, dtype=np.float32)
    k = np.ascontiguousarray(k, dtype=np.float32)
    v = np.ascontiguousarray(v, dtype=np.float32)
    L = np.asarray(valid_lens).reshape(-1).astype(np.int64)

    need = np.where(L == 0, KT, np.minimum(KT, (L + 127) // 128)).astype(np.int64)
    slot_ts, assign = _schedule(need)

    if slot_ts not in _build_cache:
        _build_cache[slot_ts] = _build(slot_ts)
    nc = _build_cache[slot_ts]

    from ml_dtypes import float8_e4m3

    qh = q.astype(np.float16)
    kh = k.astype(np.float16)
    vh = v.astype(np.float16)
    v8h = v.astype(float8_e4m3)
    kidx = np.arange(128)
    tmax = max(slot_ts)

    in_maps = []
    for c in range(NCORES):
        qT_a = np.zeros((len(slot_ts), D, QCH), np.float16)
        kT_a = np.zeros((len(slot_ts), D, tmax * 128), np.float16)
        v_a = np.zeros((len(slot_ts), tmax * 128, D), np.float16)
        sc_a = np.zeros((len(slot_ts), 128, KT), np.float32)
        bi_a = np.zeros((len(slot_ts), 128, KT), np.float32)
        for s, t in enumerate(slot_ts):
            dset = _dve_set(t)
            part = assign[c][s]
            # default: all tiles padded-off (E=0)
            for i in range(t):
                if i in dset:
                    sc_a[s, :, i] = 0.0
                    bi_a[s, :, i] = 0.0
                else:
                    sc_a[s, :, i] = 0.0
                    bi_a[s, :, i] = NEG_BIAS
            if part is None:
                continue
            b, h, t0, nt = part
            lb = int(L[b])
            qT_a[s] = qh[b, h * QCH : (h + 1) * QCH].T
            kT_a[s, :, : nt * 128] = kh[b].T[:, t0 * 128 : (t0 + nt) * 128]
            v_a[s, : nt * 128] = vh[b, t0 * 128 : (t0 + nt) * 128]
            for i in range(nt):
                g = t0 + i  # global key tile
                if lb == 0:
                    m = None  # uniform: E=1 on every key
                else:
                    m = (g * 128 + kidx < lb).astype(np.float32)
                if i in dset:
                    if m is None:
                        sc_a[s, :, i] = 0.0
                        bi_a[s, :, i] = SCHR_B + 44.5  # exactly 15360 -> E=1.0
                    else:
                        sc_a[s, :, i] = m * np.float32(SCALE * SCHR_A)
                        bi_a[s, :, i] = m * np.float32(SCHR_B + SHIFT * SCHR_A)
                else:
                    if m is None:
                        sc_a[s, :, i] = 0.0
                        bi_a[s, :, i] = 0.0
                    else:
                        sc_a[s, :, i] = m * np.float32(SCALE)
                        bi_a[s, :, i] = m * np.float32(SHIFT) + (1.0 - m) * np.float32(NEG_BIAS)
        in_maps.append(
            {"qT": qT_a, "kT": kT_a, "v": v_a, "sc": sc_a, "bi": bi_a}
        )

    res = run_bass_kernel_spmd(nc, in_maps, list(range(NCORES)))
    global last_result
    last_result = res

    acc = np.zeros((B, Q // QCH, D, QCH), np.float64)
    den = np.zeros((B, Q // QCH, QCH), np.float64)
    for c in range(NCORES):
        r = res.results[c]
        for s in range(len(slot_ts)):
            part = assign[c][s]
            if part is None:
                continue
            b, h, _, _ = part
            acc[b, h] += r["oT"][s].astype(np.float64)
            den[b, h] += r["esum"][s].astype(np.float64).sum(axis=0)
    out = np.empty((B, Q, D), np.float32)
    for b in range(B):
        for h in range(Q // QCH):
            out[b, h * QCH : (h + 1) * QCH] = (acc[b, h] / den[b, h][None, :]).T
    return out
